# revision 6
# baseline (speedup 1.0000x reference)
"""MoE MLP (dense routing, all experts) Trainium2 Bass kernel.

Problem: nn_MoEMLP_10763188044537
  S, B, D, E = 257, 64, 768, 8 ; H = 4*D = 3072 ; T = S*B = 16448 tokens.
  y[t] = sum_e softmax(x @ Wr.T + br)[t, e] * (quick_gelu(x[t] @ W1[e].T + b1[e]) @ W2[e].T + b2[e])

Sharding: expert-parallel across 8 NeuronCores. Each core holds one
expert's weights resident in SBUF (bf16), streams the replicated
activations, computes the full router softmax locally (with its own
expert's row permuted to column 0 so the same SPMD instruction stream
works on every core), weights its expert's output by the router prob,
and a ReduceScatter over the d-axis combines the partial outputs.
Host-side we only concatenate the 8 disjoint d-shards and transpose.

Everything runs in [feature, token] orientation:
  fc1:  hT[h, t]  = W1T.T @ xT   (lhsT = W1T chunk, rhs = xT chunk)
  gelu: hg = quick_gelu(hT + b1) (per-partition bias on ACT)
  fc2:  yT[d, t]  = W2T.T @ hg
  comb: yT * P  where P = ones ⊗ p  (router prob broadcast via K=1 matmul)

Compute dtype bf16 (inputs are f32; f32 matmuls run at 1/4 rate on the
PE, bf16 at full rate with f32 PSUM accumulation).
"""

import sys

if "/opt/trn_rl_repo" not in sys.path:
    sys.path.insert(0, "/opt/trn_rl_repo")

import ml_dtypes
import numpy as np

S, B, D, E = 257, 64, 768, 8
H = 4 * D
T = S * B
TT = 512           # token tile (free dim of matmuls)
CHUNK_TILES = 4    # token tiles per ReduceScatter chunk
KD = D // 128      # 6 contraction chunks for fc1 / router
KH = H // 128      # 24 contraction chunks for fc2 (and fc1 out chunks)
JD = D // 128      # 6 output-d chunks
GELU_ALPHA = 1.702

BF16 = ml_dtypes.bfloat16

# Flip after HW validation of the fused ACT table; the fallback computes
# quick_gelu as sigmoid (ACT) + add/mul (DVE) and is sim-checkable.
FUSED_GELU = True


def plan_tiles(n_tok):
    """Token tiles of TT (plus remainder) and their grouping into RS chunks.

    The last chunk is kept as small as possible (the remainder tile alone
    when there is one) so the end-of-kernel exposed ReduceScatter is short.
    """
    tiles = []
    t0 = 0
    while t0 < n_tok:
        ct = min(TT, n_tok - t0)
        tiles.append((t0, ct))
        t0 += ct
    groups = [tiles[i : i + CHUNK_TILES] for i in range(0, len(tiles), CHUNK_TILES)]
    chunks = []
    for grp in groups:
        start = grp[0][0]
        width = sum(ct for _, ct in grp)
        chunks.append((start, width))
    return tiles, groups, chunks


def build_nc(n_tok=T, fused_gelu=FUSED_GELU):
    import concourse.mybir as mybir
    import concourse.tile as tile
    from concourse import bacc

    dt = mybir.dt
    F32, BF = dt.float32, dt.bfloat16
    AF = mybir.ActivationFunctionType
    ALU = mybir.AluOpType

    tiles, groups, chunks = plan_tiles(n_tok)

    nc = bacc.Bacc("TRN2", target_bir_lowering=False, debug=False, num_devices=E)

    xT = nc.dram_tensor("xT", [D, n_tok], BF, kind="ExternalInput")
    w1t = nc.dram_tensor("w1t", [D, H], BF, kind="ExternalInput")
    w2t = nc.dram_tensor("w2t", [H, D], BF, kind="ExternalInput")
    wrt = nc.dram_tensor("wrt", [128, KD * E], BF, kind="ExternalInput")
    brc = nc.dram_tensor("brc", [E, 1], F32, kind="ExternalInput")
    b1a = nc.dram_tensor("b1a", [128, KH], F32, kind="ExternalInput")
    b1b = nc.dram_tensor("b1b", [128, KH], F32, kind="ExternalInput")
    b2s = nc.dram_tensor("b2s", [128, JD], F32, kind="ExternalInput")
    yT_out = nc.dram_tensor("yT_out", [D // E, n_tok], F32, kind="ExternalOutput")

    rg = [list(range(E))]

    with tile.TileContext(nc) as tc:
        with (
            tc.tile_pool(name="sb", bufs=1) as sbp,
            tc.tile_pool(name="wp", bufs=1) as wp,
            tc.tile_pool(name="ps", bufs=1, space="PSUM") as psp,
            tc.tile_pool(name="dram", bufs=1, space="DRAM") as dramp,
        ):
            # ---- constants / weights (resident) ----
            wrt_sb = wp.tile([128, KD * E], BF, name="wrt_sb")
            nc.sync.dma_start(wrt_sb, wrt.ap())
            brc_sb = wp.tile([E, 1], F32, name="brc_sb")
            nc.sync.dma_start(brc_sb, brc.ap())
            b1a_sb = wp.tile([128, KH], F32, name="b1a_sb")
            nc.sync.dma_start(b1a_sb, b1a.ap())
            b1b_sb = wp.tile([128, KH], F32, name="b1b_sb")
            nc.sync.dma_start(b1b_sb, b1b.ap())
            b2s_sb = wp.tile([128, JD], F32, name="b2s_sb")
            nc.sync.dma_start(b2s_sb, b2s.ap())

            ones128 = wp.tile([1, 128], BF, name="ones128")
            nc.vector.memset(ones128, 1.0)
            ones8 = wp.tile([8, 1], BF, name="ones8")
            nc.vector.memset(ones8, 1.0)

            yc_dram = []
            rs_out = []
            for g, (cstart, cwidth) in enumerate(chunks):
                yc_dram.append(
                    dramp.tile([D, cwidth], F32, name=f"yc_dram{g}", tag=f"yc{g}")
                )
                rs_out.append(
                    dramp.tile([D // E, cwidth], F32, name=f"rs_out{g}", tag=f"rs{g}")
                )

            # Weight loads go on the gpsimd (SWDGE) queue so the first token
            # tiles' activation DMAs on the sync queue aren't stuck behind
            # 9.4 MB of weights; they are only needed once fc1 of tile 0
            # starts, well after the router matmuls.
            w1_sb = []
            for k in range(KD):
                w = wp.tile([128, H], BF, name=f"w1_sb{k}", tag="w1", bufs=KD)
                nc.gpsimd.dma_start(w, xap(w1t, k * 128, 128)[:, :])
                w1_sb.append(w)
            w2_sb = []
            for k in range(KH):
                w = wp.tile([128, D], BF, name=f"w2_sb{k}", tag="w2", bufs=KH)
                nc.gpsimd.dma_start(w, xap(w2t, k * 128, 128)[:, :])
                w2_sb.append(w)

            # ---- main loop: router (tanh-based softmax; tanh shares the ACT
            # table set with the fused gelu) -> fc1 -> quick_gelu -> fc2 ->
            # bias -> prob-weight -> chunked ReduceScatter ----
            for ti, (t0, ct) in enumerate(tiles):
                g = next(
                    gi for gi, grp in enumerate(groups) if any(t == t0 for t, _ in grp)
                )
                cstart, cwidth = chunks[g]
                lo = t0 - cstart

                xts = []
                for k in range(KD):
                    xt = sbp.tile([128, ct], BF, name=f"mx{k}_{ti}", tag="x", bufs=12)
                    nc.sync.dma_start(xt, xT.ap()[k * 128 : (k + 1) * 128, t0 : t0 + ct])
                    xts.append(xt)

                # router logits for all 8 experts (this core's expert in row 0)
                lg = psp.tile([8, ct], F32, name=f"lg{ti}", tag="h", bufs=3)
                for k in range(KD):
                    nc.tensor.matmul(
                        lg,
                        lhsT=wrt_sb[:, k * 8 : (k + 1) * 8],
                        rhs=xts[k],
                        start=(k == 0),
                        stop=(k == KD - 1),
                    )
                # softmax via exp(l) = (1+tanh((l+br)/2)) / (1-tanh((l+br)/2))
                th = sbp.tile([8, ct], F32, name=f"th{ti}", tag="th", bufs=3)
                nc.scalar.activation(th, lg, AF.Tanh, bias=brc_sb, scale=0.5)
                num = sbp.tile([8, ct], F32, name=f"num{ti}", tag="num", bufs=3)
                nc.vector.tensor_scalar_add(num, th, 1.0)
                den = sbp.tile([8, ct], F32, name=f"den{ti}", tag="den", bufs=3)
                nc.vector.tensor_scalar(
                    den, th, 1.0, -1.0, op0=ALU.subtract, op1=ALU.mult
                )
                rd = sbp.tile([8, ct], F32, name=f"rd{ti}", tag="rd", bufs=3)
                nc.vector.reciprocal(rd, den)
                ex = sbp.tile([8, ct], BF, name=f"ex{ti}", tag="ex", bufs=3)
                nc.vector.tensor_tensor(ex, num, rd, op=ALU.mult)
                sm = psp.tile([1, ct], F32, name=f"sm{ti}", tag="P", bufs=2)
                nc.tensor.matmul(sm, lhsT=ones8, rhs=ex, start=True, stop=True)
                rc = sbp.tile([1, ct], F32, name=f"rc{ti}", tag="rc", bufs=3)
                nc.vector.reciprocal(rc, sm)
                pp = sbp.tile([1, ct], BF, name=f"pp{ti}", tag="pp", bufs=3)
                nc.vector.tensor_tensor(pp, ex[0:1, :], rc, op=ALU.mult)
                Pp = psp.tile([128, ct], F32, name=f"Pp{ti}", tag="P", bufs=2)
                nc.tensor.matmul(Pp, lhsT=ones128, rhs=pp, start=True, stop=True)

                hgs = []
                for m in range(KH):
                    hp = psp.tile([128, ct], F32, name=f"hp{ti}_{m}", tag="h", bufs=3)
                    for k in range(KD):
                        nc.tensor.matmul(
                            hp,
                            lhsT=w1_sb[k][:, m * 128 : (m + 1) * 128],
                            rhs=xts[k],
                            start=(k == 0),
                            stop=(k == KD - 1),
                        )
                    hg = sbp.tile(
                        [128, ct], BF, name=f"hg{ti}_{m}", tag=f"hg{m}", bufs=2
                    )
                    if fused_gelu:
                        nc.scalar.activation(
                            hg,
                            hp,
                            AF.Gelu_apprx_sigmoid,
                            bias=b1a_sb[:, m : m + 1],
                            scale=1.0,
                        )
                    else:
                        sg = sbp.tile([128, ct], F32, name=f"sg{ti}_{m}", tag="sg", bufs=3)
                        nc.scalar.activation(
                            sg,
                            hp,
                            AF.Sigmoid,
                            bias=b1b_sb[:, m : m + 1],
                            scale=GELU_ALPHA,
                        )
                        zz = sbp.tile([128, ct], F32, name=f"zz{ti}_{m}", tag="zz", bufs=3)
                        nc.vector.tensor_scalar_add(zz, hp, b1a_sb[:, m : m + 1])
                        nc.vector.tensor_tensor(hg, zz, sg, op=ALU.mult)
                    hgs.append(hg)

                for j in range(JD):
                    yp = psp.tile([128, ct], F32, name=f"yp{ti}_{j}", tag="y", bufs=2)
                    for k in range(KH):
                        nc.tensor.matmul(
                            yp,
                            lhsT=w2_sb[k][:, j * 128 : (j + 1) * 128],
                            rhs=hgs[k],
                            start=(k == 0),
                            stop=(k == KH - 1),
                        )
                    yb = sbp.tile([128, ct], F32, name=f"yb{ti}_{j}", tag="yb", bufs=3)
                    nc.scalar.activation(yb, yp, AF.Identity, bias=b2s_sb[:, j : j + 1])
                    yw = sbp.tile([128, ct], F32, name=f"yw{ti}_{j}", tag="yw", bufs=4)
                    nc.vector.tensor_tensor(yw, yb, Pp, op=ALU.mult)
                    nc.sync.dma_start(
                        yc_dram[g][j * 128 : (j + 1) * 128, lo : lo + ct], yw
                    )

                if (t0, ct) == groups[g][-1]:
                    nc.gpsimd.collective_compute(
                        "ReduceScatter",
                        ALU.add,
                        replica_groups=rg,
                        ins=[yc_dram[g].opt()],
                        outs=[rs_out[g].opt()],
                    )
                    nc.sync.dma_start(
                        yT_out.ap()[:, cstart : cstart + cwidth], rs_out[g]
                    )

    nc.compile()
    return nc


def xap(handle, row0, nrows):
    """Row-slice helper for 2D DRAM tensors."""
    return handle.ap()[row0 : row0 + nrows, :]


def prep_inputs(x, W1, b1, W2, b2, Wr, br):
    """Host-side shard prep. Returns in_maps for the 8 cores."""
    x = np.asarray(x, dtype=np.float32)
    s, b, d = x.shape
    xf = x.reshape(s * b, d)
    xT_bf = np.ascontiguousarray(xf.T).astype(BF16)  # [D, T]

    W1 = np.asarray(W1, dtype=np.float32)
    W2 = np.asarray(W2, dtype=np.float32)
    b1 = np.asarray(b1, dtype=np.float32)
    b2 = np.asarray(b2, dtype=np.float32)
    Wr = np.asarray(Wr, dtype=np.float32)
    br = np.asarray(br, dtype=np.float32)

    in_maps = []
    for e in range(E):
        perm = [e] + [i for i in range(E) if i != e]
        w1t = np.ascontiguousarray(W1[e].T).astype(BF16)  # [D, H]
        w2t = np.ascontiguousarray(W2[e].T).astype(BF16)  # [H, D]
        wrt_p = np.ascontiguousarray(Wr[perm].T)  # [D, E]
        wrt = np.ascontiguousarray(
            wrt_p.reshape(KD, 128, E).transpose(1, 0, 2).reshape(128, KD * E)
        ).astype(BF16)
        brc = np.ascontiguousarray(0.5 * br[perm].reshape(E, 1)).astype(np.float32)
        b1a = np.ascontiguousarray(b1[e].reshape(KH, 128).T)  # [128, KH] f32
        b1b = np.ascontiguousarray(GELU_ALPHA * b1a)
        b2sa = np.ascontiguousarray(b2[e].reshape(JD, 128).T)  # [128, JD] f32
        in_maps.append(
            {
                "xT": xT_bf,
                "w1t": w1t,
                "w2t": w2t,
                "wrt": wrt,
                "brc": brc,
                "b1a": b1a,
                "b1b": b1b,
                "b2s": b2sa,
            }
        )
    return in_maps


def assemble_output(per_core_yT, s, b):
    yT = np.concatenate(per_core_yT, axis=0)  # [D, T]
    return np.ascontiguousarray(yT.T).reshape(s, b, D).astype(np.float32)


_NC_CACHE = {}


def get_nc(n_tok=T, fused_gelu=FUSED_GELU):
    key = (n_tok, fused_gelu)
    if key not in _NC_CACHE:
        _NC_CACHE[key] = build_nc(n_tok, fused_gelu)
    return _NC_CACHE[key]


def kernel(x, W1, b1, W2, b2, Wr, br, trace=False):
    from concourse.bass_utils import run_bass_kernel_spmd

    x = np.asarray(x, dtype=np.float32)
    s, b, d = x.shape
    nc = get_nc(n_tok=s * b)
    in_maps = prep_inputs(x, W1, b1, W2, b2, Wr, br)
    res = run_bass_kernel_spmd(nc, in_maps, core_ids=list(range(E)), trace=trace)
    out = assemble_output([res.results[e]["yT_out"] for e in range(E)], s, b)
    if trace:
        kernel.last_result = res
    return out


# revision 11
# speedup vs baseline: 1.0682x; 1.0682x over previous
"""MoE MLP (dense routing, all experts) Trainium2 Bass kernel.

Problem: nn_MoEMLP_10763188044537
  S, B, D, E = 257, 64, 768, 8 ; H = 4*D = 3072 ; T = S*B = 16448 tokens.
  y[t] = sum_e softmax(x @ Wr.T + br)[t, e] * (quick_gelu(x[t] @ W1[e].T + b1[e]) @ W2[e].T + b2[e])

Sharding: expert-parallel across 8 NeuronCores. Each core holds one
expert's weights resident in SBUF (bf16), streams the replicated
activations, computes the full router softmax locally (with its own
expert's row permuted to column 0 so the same SPMD instruction stream
works on every core), weights its expert's output by the router prob,
and a ReduceScatter over the d-axis combines the partial outputs.
Host-side we only concatenate the 8 disjoint d-shards and transpose.

Everything runs in [feature, token] orientation:
  fc1:  hT[h, t]  = W1T.T @ xT   (lhsT = W1T chunk, rhs = xT chunk)
  gelu: hg = quick_gelu(hT + b1) (per-partition bias on ACT)
  fc2:  yT[d, t]  = W2T.T @ hg
  comb: yT * P  where P = ones ⊗ p  (router prob broadcast via K=1 matmul)

Compute dtype bf16 (inputs are f32; f32 matmuls run at 1/4 rate on the
PE, bf16 at full rate with f32 PSUM accumulation).
"""

import sys

if "/opt/trn_rl_repo" not in sys.path:
    sys.path.insert(0, "/opt/trn_rl_repo")

import ml_dtypes
import numpy as np

S, B, D, E = 257, 64, 768, 8
H = 4 * D
T = S * B
TT = 512           # token tile (free dim of matmuls)
CHUNK_TILES = 4    # token tiles per ReduceScatter chunk
KD = D // 128      # 6 contraction chunks for fc1 / router
KH = H // 128      # 24 contraction chunks for fc2 (and fc1 out chunks)
JD = D // 128      # 6 output-d chunks
GELU_ALPHA = 1.702

BF16 = ml_dtypes.bfloat16

# Flip after HW validation of the fused ACT table; the fallback computes
# quick_gelu as sigmoid (ACT) + add/mul (DVE) and is sim-checkable.
FUSED_GELU = True


def plan_tiles(n_tok):
    """Token tiles of TT (plus remainder) and their grouping into RS chunks.

    The last chunk is kept as small as possible (the remainder tile alone
    when there is one) so the end-of-kernel exposed ReduceScatter is short.
    """
    tiles = []
    t0 = 0
    while t0 < n_tok:
        ct = min(TT, n_tok - t0)
        tiles.append((t0, ct))
        t0 += ct
    # Taper the chunk sizes at the end: the ReduceScatters execute serially,
    # so the exposed tail is (last chunks' RS chain) minus the compute that
    # still overlaps it. Groups of 4 tiles, then 2+2 for the last 4 full
    # tiles, then the remainder tile alone.
    full = [t for t in tiles if t[1] == TT]
    rest = [t for t in tiles if t[1] != TT]
    groups = []
    i = 0
    while len(full) - i > 4:
        take = min(CHUNK_TILES, len(full) - i - 4)
        groups.append(full[i : i + take])
        i += take
    while i < len(full):
        groups.append(full[i : i + 2])
        i += 2
    if rest:
        groups.append(rest)
    chunks = []
    for grp in groups:
        start = grp[0][0]
        width = sum(ct for _, ct in grp)
        chunks.append((start, width))
    return tiles, groups, chunks


def build_nc(n_tok=T, fused_gelu=FUSED_GELU):
    import concourse.mybir as mybir
    import concourse.tile as tile
    from concourse import bacc

    dt = mybir.dt
    F32, BF = dt.float32, dt.bfloat16
    AF = mybir.ActivationFunctionType
    ALU = mybir.AluOpType

    tiles, groups, chunks = plan_tiles(n_tok)

    nc = bacc.Bacc("TRN2", target_bir_lowering=False, debug=False, num_devices=E)

    xT = nc.dram_tensor("xT", [D, n_tok], BF, kind="ExternalInput")
    w1t = nc.dram_tensor("w1t", [D, H], BF, kind="ExternalInput")
    w2t = nc.dram_tensor("w2t", [H, D], BF, kind="ExternalInput")
    wrt = nc.dram_tensor("wrt", [128, KD * E], BF, kind="ExternalInput")
    brc = nc.dram_tensor("brc", [E, 1], F32, kind="ExternalInput")
    b1a = nc.dram_tensor("b1a", [128, KH], F32, kind="ExternalInput")
    b1b = nc.dram_tensor("b1b", [128, KH], F32, kind="ExternalInput")
    b2s = nc.dram_tensor("b2s", [128, JD], F32, kind="ExternalInput")
    yT_out = nc.dram_tensor("yT_out", [D // E, n_tok], F32, kind="ExternalOutput")

    rg = [list(range(E))]

    with tile.TileContext(nc) as tc:
        with (
            tc.tile_pool(name="sb", bufs=1) as sbp,
            tc.tile_pool(name="wp", bufs=1) as wp,
            tc.tile_pool(name="ps", bufs=1, space="PSUM") as psp,
            tc.tile_pool(name="dram", bufs=1, space="DRAM") as dramp,
        ):
            # ---- constants / weights (resident) ----
            wrt_sb = wp.tile([128, KD * E], BF, name="wrt_sb")
            nc.sync.dma_start(wrt_sb, wrt.ap())
            brc_sb = wp.tile([E, 1], F32, name="brc_sb")
            nc.sync.dma_start(brc_sb, brc.ap())
            b1a_sb = wp.tile([128, KH], F32, name="b1a_sb")
            nc.sync.dma_start(b1a_sb, b1a.ap())
            b1b_sb = wp.tile([128, KH], F32, name="b1b_sb")
            nc.sync.dma_start(b1b_sb, b1b.ap())
            b2s_sb = wp.tile([128, JD], F32, name="b2s_sb")
            nc.sync.dma_start(b2s_sb, b2s.ap())

            ones128 = wp.tile([1, 128], BF, name="ones128")
            nc.vector.memset(ones128, 1.0)
            ones8 = wp.tile([8, 1], BF, name="ones8")
            nc.vector.memset(ones8, 1.0)

            yc_dram = []
            rs_out = []
            for g, (cstart, cwidth) in enumerate(chunks):
                yc_dram.append(
                    dramp.tile([D, cwidth], F32, name=f"yc_dram{g}", tag=f"yc{g}")
                )
                rs_out.append(
                    dramp.tile([D // E, cwidth], F32, name=f"rs_out{g}", tag=f"rs{g}")
                )

            # Weight loads go on the gpsimd (SWDGE) queue so the first token
            # tiles' activation DMAs on the sync queue aren't stuck behind
            # 9.4 MB of weights; they are only needed once fc1 of tile 0
            # starts, well after the router matmuls.
            w1_sb = []
            for k in range(KD):
                w = wp.tile([128, H], BF, name=f"w1_sb{k}", tag="w1", bufs=KD)
                nc.gpsimd.dma_start(w, xap(w1t, k * 128, 128)[:, :])
                w1_sb.append(w)
            w2_sb = []
            for k in range(KH):
                w = wp.tile([128, D], BF, name=f"w2_sb{k}", tag="w2", bufs=KH)
                nc.gpsimd.dma_start(w, xap(w2t, k * 128, 128)[:, :])
                w2_sb.append(w)

            # ---- main loop: router (tanh-based softmax; tanh shares the ACT
            # table set with the fused gelu) -> fc1 -> quick_gelu -> fc2 ->
            # bias -> prob-weight -> chunked ReduceScatter ----
            for ti, (t0, ct) in enumerate(tiles):
                g = next(
                    gi for gi, grp in enumerate(groups) if any(t == t0 for t, _ in grp)
                )
                cstart, cwidth = chunks[g]
                lo = t0 - cstart

                xts = []
                for k in range(KD):
                    xt = sbp.tile([128, ct], BF, name=f"mx{k}_{ti}", tag="x", bufs=16)
                    nc.sync.dma_start(xt, xT.ap()[k * 128 : (k + 1) * 128, t0 : t0 + ct])
                    xts.append(xt)

                # router logits for all 8 experts (this core's expert in row 0)
                lg = psp.tile([8, ct], F32, name=f"lg{ti}", tag="h", bufs=3)
                for k in range(KD):
                    nc.tensor.matmul(
                        lg,
                        lhsT=wrt_sb[:, k * 8 : (k + 1) * 8],
                        rhs=xts[k],
                        start=(k == 0),
                        stop=(k == KD - 1),
                    )
                # softmax via exp(l) = (1+tanh((l+br)/2)) / (1-tanh((l+br)/2))
                th = sbp.tile([8, ct], F32, name=f"th{ti}", tag="th", bufs=3)
                nc.scalar.activation(th, lg, AF.Tanh, bias=brc_sb, scale=0.5)
                num = sbp.tile([8, ct], F32, name=f"num{ti}", tag="num", bufs=3)
                nc.vector.tensor_scalar_add(num, th, 1.0)
                den = sbp.tile([8, ct], F32, name=f"den{ti}", tag="den", bufs=3)
                nc.vector.tensor_scalar(
                    den, th, 1.0, -1.0, op0=ALU.subtract, op1=ALU.mult
                )
                rd = sbp.tile([8, ct], F32, name=f"rd{ti}", tag="rd", bufs=3)
                nc.vector.reciprocal(rd, den)
                ex = sbp.tile([8, ct], BF, name=f"ex{ti}", tag="ex", bufs=3)
                nc.vector.tensor_tensor(ex, num, rd, op=ALU.mult)

                hgs = []
                for m in range(KH):
                    hp = psp.tile([128, ct], F32, name=f"hp{ti}_{m}", tag="h", bufs=3)
                    for k in range(KD):
                        nc.tensor.matmul(
                            hp,
                            lhsT=w1_sb[k][:, m * 128 : (m + 1) * 128],
                            rhs=xts[k],
                            start=(k == 0),
                            stop=(k == KD - 1),
                        )
                    hg = sbp.tile(
                        [128, ct], BF, name=f"hg{ti}_{m}", tag=f"hg{m}", bufs=2
                    )
                    if fused_gelu:
                        nc.scalar.activation(
                            hg,
                            hp,
                            AF.Gelu_apprx_sigmoid,
                            bias=b1a_sb[:, m : m + 1],
                            scale=1.0,
                        )
                    else:
                        sg = sbp.tile([128, ct], F32, name=f"sg{ti}_{m}", tag="sg", bufs=3)
                        nc.scalar.activation(
                            sg,
                            hp,
                            AF.Sigmoid,
                            bias=b1b_sb[:, m : m + 1],
                            scale=GELU_ALPHA,
                        )
                        zz = sbp.tile([128, ct], F32, name=f"zz{ti}_{m}", tag="zz", bufs=3)
                        nc.vector.tensor_scalar_add(zz, hp, b1a_sb[:, m : m + 1])
                        nc.vector.tensor_tensor(hg, zz, sg, op=ALU.mult)
                    hgs.append(hg)

                    # The softmax sum matmul goes a few m-chunks into fc1 in
                    # PE program order (its DVE-chain input is done by then),
                    # and the prob-broadcast matmul at the end of fc1 (its
                    # input pp needs two more DVE ops after sm). This way the
                    # PE never stalls on the router's DVE chain.
                    if m == 3:
                        sm = psp.tile([1, ct], F32, name=f"sm{ti}", tag="P", bufs=2)
                        nc.tensor.matmul(sm, lhsT=ones8, rhs=ex, start=True, stop=True)
                        rc = sbp.tile([1, ct], F32, name=f"rc{ti}", tag="rc", bufs=3)
                        nc.vector.reciprocal(rc, sm)
                        pp = sbp.tile([1, ct], BF, name=f"pp{ti}", tag="pp", bufs=3)
                        nc.vector.tensor_tensor(pp, ex[0:1, :], rc, op=ALU.mult)

                Pp = psp.tile([128, ct], F32, name=f"Pp{ti}", tag="P", bufs=2)
                nc.tensor.matmul(Pp, lhsT=ones128, rhs=pp, start=True, stop=True)

                for j in range(JD):
                    yp = psp.tile([128, ct], F32, name=f"yp{ti}_{j}", tag="y", bufs=2)
                    for k in range(KH):
                        nc.tensor.matmul(
                            yp,
                            lhsT=w2_sb[k][:, j * 128 : (j + 1) * 128],
                            rhs=hgs[k],
                            start=(k == 0),
                            stop=(k == KH - 1),
                        )
                    yb = sbp.tile([128, ct], F32, name=f"yb{ti}_{j}", tag="yb", bufs=3)
                    nc.scalar.activation(yb, yp, AF.Identity, bias=b2s_sb[:, j : j + 1])
                    yw = sbp.tile([128, ct], F32, name=f"yw{ti}_{j}", tag="yw", bufs=4)
                    nc.vector.tensor_tensor(yw, yb, Pp, op=ALU.mult)
                    nc.sync.dma_start(
                        yc_dram[g][j * 128 : (j + 1) * 128, lo : lo + ct], yw
                    )

                if (t0, ct) == groups[g][-1]:
                    nc.gpsimd.collective_compute(
                        "ReduceScatter",
                        ALU.add,
                        replica_groups=rg,
                        ins=[yc_dram[g].opt()],
                        outs=[rs_out[g].opt()],
                    )
                    nc.sync.dma_start(
                        yT_out.ap()[:, cstart : cstart + cwidth], rs_out[g]
                    )

    nc.compile()
    return nc


def xap(handle, row0, nrows):
    """Row-slice helper for 2D DRAM tensors."""
    return handle.ap()[row0 : row0 + nrows, :]


def prep_inputs(x, W1, b1, W2, b2, Wr, br):
    """Host-side shard prep. Returns in_maps for the 8 cores."""
    x = np.asarray(x, dtype=np.float32)
    s, b, d = x.shape
    xf = x.reshape(s * b, d)
    xT_bf = np.ascontiguousarray(xf.T).astype(BF16)  # [D, T]

    W1 = np.asarray(W1, dtype=np.float32)
    W2 = np.asarray(W2, dtype=np.float32)
    b1 = np.asarray(b1, dtype=np.float32)
    b2 = np.asarray(b2, dtype=np.float32)
    Wr = np.asarray(Wr, dtype=np.float32)
    br = np.asarray(br, dtype=np.float32)

    in_maps = []
    for e in range(E):
        perm = [e] + [i for i in range(E) if i != e]
        w1t = np.ascontiguousarray(W1[e].T).astype(BF16)  # [D, H]
        w2t = np.ascontiguousarray(W2[e].T).astype(BF16)  # [H, D]
        wrt_p = np.ascontiguousarray(Wr[perm].T)  # [D, E]
        wrt = np.ascontiguousarray(
            wrt_p.reshape(KD, 128, E).transpose(1, 0, 2).reshape(128, KD * E)
        ).astype(BF16)
        brc = np.ascontiguousarray(0.5 * br[perm].reshape(E, 1)).astype(np.float32)
        b1a = np.ascontiguousarray(b1[e].reshape(KH, 128).T)  # [128, KH] f32
        b1b = np.ascontiguousarray(GELU_ALPHA * b1a)
        b2sa = np.ascontiguousarray(b2[e].reshape(JD, 128).T)  # [128, JD] f32
        in_maps.append(
            {
                "xT": xT_bf,
                "w1t": w1t,
                "w2t": w2t,
                "wrt": wrt,
                "brc": brc,
                "b1a": b1a,
                "b1b": b1b,
                "b2s": b2sa,
            }
        )
    return in_maps


def assemble_output(per_core_yT, s, b):
    yT = np.concatenate(per_core_yT, axis=0)  # [D, T]
    return np.ascontiguousarray(yT.T).reshape(s, b, D).astype(np.float32)


_NC_CACHE = {}


def get_nc(n_tok=T, fused_gelu=FUSED_GELU):
    key = (n_tok, fused_gelu)
    if key not in _NC_CACHE:
        _NC_CACHE[key] = build_nc(n_tok, fused_gelu)
    return _NC_CACHE[key]


def kernel(x, W1, b1, W2, b2, Wr, br, trace=False):
    from concourse.bass_utils import run_bass_kernel_spmd

    x = np.asarray(x, dtype=np.float32)
    s, b, d = x.shape
    nc = get_nc(n_tok=s * b)
    in_maps = prep_inputs(x, W1, b1, W2, b2, Wr, br)
    res = run_bass_kernel_spmd(nc, in_maps, core_ids=list(range(E)), trace=trace)
    out = assemble_output([res.results[e]["yT_out"] for e in range(E)], s, b)
    if trace:
        kernel.last_result = res
    return out
